# revision 29
# baseline (speedup 1.0000x reference)
"""Trainium2 Bass kernel for nn_BaseRuleLearner (pair-decomposition design).

Math (per batch b, rule i; perm p=(a,b,c) distinct):
  score = F01(a,b)+F02(a,c)+F12(b,c)+g0(a)+g1(b)+g2(c), where
  Ff(x,y)  = BM[nm,x,y]+BM[mn,y,x]  for (n,m) in [(0,1),(0,2),(1,2)]
  gv(l)    = UM[v,l]+BM[vv,l,l]
  out      = softmax_i(min_p score) @ one_hot([0,0,1,1])

Stage 1 (PE): one matmul per unordered pair {j<k} with k-dim = (w,e):
  w=0 rows hold Bf[b,j,k,:], w=1 rows Bf[b,k,j,:].  Weight cols (i, d=2f+o)
  combine rb[n,m]/rb[m,n] so each output row is a full Ff value for one
  orientation: psum rows (i,d)=24 at psum partition offset 32*sl, packing
  4 pairs per [128,512] psum tile (7 tiles).  Unary+diag: 2 matmuls per
  lp-slot accumulate ru and rb-diag into one [128,512] psum tile (g-rows).

Evac: 8 copies [128,512] fp32->bf16 (vector/scalar/gpsimd) to staging.

Assembly (12 DMAs): per i, scatter staging rows into k-major Q tiles
  qt0/qt1 [96, (i,b)]: off-diag k-row r=q*6+d (chunk0 q<16), g-rows
  r=168+v*8+l (chunk1 local 72..95).

Stage 2 (PE): per (bt,i): 2 matmuls (k=96 chunks) accumulate
  psum[128b, 336p] with 0/1 gather matrix G; min over p (vector/gpsimd),
  softmax over i, pair-sum into [128,4], one gathered output DMA.
"""

import itertools
import numpy as np

B, O, E = 4096, 8, 64
I, V = 4, 3
P = 336
N_CORES = 8
BC = B // N_CORES            # 512 batch per core
NPAIR = 28                   # unordered pairs {j<k}
JBS = BC + 16                # padded block stride in staging tiles
FMAP = [(0, 1), (0, 2), (1, 2)]

_PERM = np.array(list(itertools.permutations(range(O), V)), dtype=np.int32)
_PAIRS = [(j, k) for j in range(O) for k in range(j + 1, O)]
_QIDX = {pr: q for q, pr in enumerate(_PAIRS)}

_CACHED = {}


def _build_g():
    """Gather matrix G [192, P] in k-row order r=q*6+2f+o (off-diag),
    r=168+v*8+l (g-rows); returned packed as [96, 2*P] (chunk0|chunk1)."""
    g = np.zeros((108, 2 * P), np.float32)
    for p, (a, b, c) in enumerate(_PERM):
        for f, (x, y) in enumerate(((a, b), (a, c), (b, c))):
            q, o = (_QIDX[(x, y)], 0) if x < y else (_QIDX[(y, x)], 1)
            d = f * 2 + o
            sl2, pg = q % 2, q // 2
            g[d * 14 + pg, sl2 * P + p] += 1.0
        for v, x in ((0, a), (1, b), (2, c)):
            g[84 + (v * 2 + x % 2) * 4 + x // 2, P + p] += 1.0
    return g  # [108, 672]: chunk0 rows 0:84 | chunk1 rows 0:108


def _build_w(rule_unary, rule_binary):
    """Weights [128, 88]: cols 0:24 off-diag (i*6+2f+o), 24:56 unary
    (i*8+v*2+s, 2 pad cols per i), 56:88 diag (same col order)."""
    ru = np.asarray(rule_unary, np.float32)
    rb = np.asarray(rule_binary, np.float32)
    w = np.zeros((128, 88), np.float32)
    for i in range(I):
        for f, (n, m) in enumerate(FMAP):
            for o in range(2):
                c = i * 6 + f * 2 + o
                w[0:64, c] = rb[i, n, m] if o == 0 else rb[i, m, n]
                w[64:128, c] = rb[i, m, n] if o == 0 else rb[i, n, m]
        for v in range(V):
            for s in range(2):
                c = i * 8 + v * 2 + s
                w[s * 64:(s + 1) * 64, 24 + c] = ru[i, v]
                w[s * 64:(s + 1) * 64, 56 + c] = rb[i, v, v]
    return w


def _build_module():
    import concourse.tile as tile
    from concourse import bacc, mybir

    FP = mybir.dt.float32
    BF = mybir.dt.bfloat16
    X = mybir.AxisListType.X
    nc = bacc.Bacc("TRN2", target_bir_lowering=False, debug=False)

    ao = nc.dram_tensor("ao", [128, NPAIR * BC], BF, kind="ExternalInput")
    ag = nc.dram_tensor("ag", [128, 8 * BC], BF, kind="ExternalInput")
    w = nc.dram_tensor("w", [128, 88], BF, kind="ExternalInput")
    gm = nc.dram_tensor("gm", [108, 2 * P], BF, kind="ExternalInput")
    out = nc.dram_tensor("out", [BC, 4], FP, kind="ExternalOutput")

    NBT = BC // 128

    with tile.TileContext(nc) as tc:
        with (
            tc.tile_pool(name="wpool", bufs=1) as wpool,
            tc.tile_pool(name="xpool", bufs=1) as xpool,
            tc.tile_pool(name="sgpool", bufs=1) as sgpool,
            tc.tile_pool(name="qpool", bufs=1) as qpool,
            tc.tile_pool(name="mpool", bufs=2) as mpool,
            tc.tile_pool(name="pod", bufs=3, space="PSUM") as pod,
            tc.tile_pool(name="pgp", bufs=2, space="PSUM") as pgp,
            tc.tile_pool(name="pss", bufs=3, space="PSUM") as pss,
        ):
            # ---- input DMAs (sync queue, FIFO) ----
            w_sb = wpool.tile([128, 88], BF, tag="w")
            nc.sync.dma_start(w_sb[:], w.ap()[:])
            ag_sb = xpool.tile([128, 8 * BC], BF, tag="ag")
            nc.sync.dma_start(ag_sb[:], ag.ap()[:])
            g_sb = wpool.tile([108, 2 * P], BF, tag="g")
            nc.sync.dma_start(g_sb[:], gm.ap()[:])
            ao_sb = xpool.tile([128, NPAIR * BC], BF, tag="ao")
            for ch in range(7):
                nc.sync.dma_start(
                    ao_sb[:, ch * 4 * BC:(ch + 1) * 4 * BC],
                    ao.ap()[:, ch * 4 * BC:(ch + 1) * 4 * BC],
                )

            sg_od = sgpool.tile([128, 14 * JBS], BF, tag="sgod")
            sg_g = sgpool.tile([32, 4 * JBS], BF, tag="sgg")
            qt0 = qpool.tile([84, 4 * BC], BF, tag="qt0")
            qt1 = qpool.tile([108, 4 * BC], BF, tag="qt1")

            # ---- stage 1: g (unary+diag), 2 accumulating matmuls/tile ----
            for lp in range(4):
                ps_g = pgp.tile([32, BC], FP, tag="pg")
                nc.tensor.matmul(
                    ps_g[:], w_sb[:, 24:56], ag_sb[:, lp * BC:(lp + 1) * BC],
                    start=True, stop=False,
                )
                nc.tensor.matmul(
                    ps_g[:], w_sb[:, 56:88],
                    ag_sb[:, (4 + lp) * BC:(5 + lp) * BC],
                    start=False, stop=True,
                )
                eng = nc.vector if lp % 2 == 0 else nc.scalar
                if lp % 2 == 0:
                    nc.vector.tensor_copy(
                        sg_g[:, lp * JBS:lp * JBS + BC], ps_g[:]
                    )
                else:
                    nc.scalar.copy(sg_g[:, lp * JBS:lp * JBS + BC], ps_g[:])

            # ---- stage 1: off-diag pairs, 2 per psum tile (slots 0/64) ----
            for pg in range(14):
                ps = pod.tile([128, BC], FP, tag="pod")
                for sl in range(2):
                    q = pg * 2 + sl
                    nc.tensor.matmul(
                        ps[64 * sl:64 * sl + 24, :],
                        w_sb[:, 0:24],
                        ao_sb[:, q * BC:(q + 1) * BC],
                        start=True, stop=True, tile_position=(0, 64 * sl),
                    )
                dst = sg_od[:, pg * JBS:pg * JBS + BC]
                if pg % 2 == 0:
                    nc.vector.tensor_copy(dst, ps[:])
                else:
                    nc.scalar.copy(dst, ps[:])

            # ---- assembly: scatter staging -> k-major qt0/qt1 ----
            # chunk0 (slot 0): k-row r = d*14 + pg -> qt0
            # chunk1 (slot 1): r = d*14 + pg, plus g rows 84 + (v,s)*4 + lp
            # src APs: ONE partition dim, outermost
            sgv = sg_od[:].rearrange("(s r) (a m) -> s r a m", s=2, m=JBS)
            qeng = [nc.sync, nc.gpsimd, nc.sync, nc.gpsimd]
            for i in range(I):
                qeng[i].dma_start(
                    qt0[0:84, i * BC:(i + 1) * BC],
                    sgv[0, i * 6:i * 6 + 6, 0:14, 0:BC],
                )
                qeng[(i + 1) % 4].dma_start(
                    qt1[0:84, i * BC:(i + 1) * BC],
                    sgv[1, i * 6:i * 6 + 6, 0:14, 0:BC],
                )
                srcg = (
                    sg_g[i * 8:i * 8 + 6, :]
                    .rearrange("p (a m) -> p a m", m=JBS)[:, :, 0:BC]
                )
                nc.scalar.dma_start(
                    qt1[84:108, i * BC:(i + 1) * BC], srcg
                )

            # ---- stage 2: scores + min + softmax ----
            fin = mpool.tile([128, 4 * NBT], FP, tag="fin", bufs=1)
            for bt in range(NBT):
                merged = mpool.tile([128, 4], FP, tag="m")
                for i in range(I):
                    sc = pss.tile([128, P], FP, tag="sc")
                    col = i * BC + bt * 128
                    nc.tensor.matmul(
                        sc[:], qt0[:, col:col + 128], g_sb[0:84, 0:P],
                        start=True, stop=False,
                    )
                    nc.tensor.matmul(
                        sc[:], qt1[:, col:col + 128], g_sb[:, P:2 * P],
                        start=False, stop=True,
                    )
                    nc.vector.tensor_reduce(
                        merged[:, i:i + 1], sc[:], axis=X,
                        op=mybir.AluOpType.min,
                    )
                mx = mpool.tile([128, 1], FP, tag="mx")
                nc.vector.tensor_reduce(
                    mx[:], merged[:], axis=X, op=mybir.AluOpType.max
                )
                sh = mpool.tile([128, 4], FP, tag="sh")
                nc.vector.tensor_scalar_sub(sh[:], merged[:], mx[:])
                ex = mpool.tile([128, 4], FP, tag="ex")
                sm = mpool.tile([128, 1], FP, tag="sm")
                nc.scalar.activation(
                    ex[:], sh[:], mybir.ActivationFunctionType.Exp,
                    accum_out=sm[:],
                )
                rc = mpool.tile([128, 1], FP, tag="rc")
                nc.vector.reciprocal(rc[:], sm[:])
                pr = mpool.tile([128, 4], FP, tag="pr")
                nc.vector.tensor_scalar_mul(pr[:], ex[:], rc[:])
                pr3 = pr[:].rearrange("p (a b) -> p a b", b=2)
                nc.vector.tensor_add(
                    fin[:, bt * 4:bt * 4 + 2], pr3[:, :, 0], pr3[:, :, 1]
                )
                nc.vector.memset(fin[:, bt * 4 + 2:bt * 4 + 4], 0.0)
            outv = out.ap().rearrange("(a p) m -> p a m", p=128)
            nc.sync.dma_start(outv, fin[:].rearrange("p (a m) -> p a m", a=NBT))

    nc.compile()
    return nc


def _get_module():
    if "nc" not in _CACHED:
        _CACHED["nc"] = _build_module()
    return _CACHED["nc"]


def _host_inputs(unary_feats, binary_feats, rule_unary, rule_binary):
    import ml_dtypes

    bf16 = ml_dtypes.bfloat16
    uf = np.asarray(unary_feats, np.float32)
    bf = np.asarray(binary_feats, np.float32)

    w = _build_w(rule_unary, rule_binary).astype(bf16)
    g = _build_g().astype(bf16)
    jj = np.array([p[0] for p in _PAIRS])
    kk = np.array([p[1] for p in _PAIRS])

    in_maps = []
    for c in range(N_CORES):
        bfc = bf[c * BC:(c + 1) * BC]                    # [BC, O, O, E]
        ufc = uf[c * BC:(c + 1) * BC]                    # [BC, O, E]
        ao = np.empty((128, NPAIR * BC), np.float32)
        ao[0:64] = bfc[:, jj, kk, :].transpose(2, 1, 0).reshape(64, -1)
        ao[64:128] = bfc[:, kk, jj, :].transpose(2, 1, 0).reshape(64, -1)
        # ag: unary blocks (lp) then diag blocks (q); rows (s, e)
        au = ufc.reshape(BC, 4, 2, E).transpose(2, 3, 1, 0).reshape(128, -1)
        dg = bfc[:, np.arange(O), np.arange(O), :]       # [BC, O, E]
        ad = dg.reshape(BC, 4, 2, E).transpose(2, 3, 1, 0).reshape(128, -1)
        ag = np.concatenate([au, ad], axis=1)
        in_maps.append({
            "ao": ao.astype(bf16), "ag": ag.astype(bf16), "w": w, "gm": g,
        })
    return in_maps


TRACE = False  # set True (e.g. from test.py) to capture an NTFF profile


def kernel(unary_feats, binary_feats, rule_unary, rule_binary):
    from concourse.bass_utils import run_bass_kernel_spmd

    nc = _get_module()
    in_maps = _host_inputs(unary_feats, binary_feats, rule_unary, rule_binary)
    res = run_bass_kernel_spmd(
        nc, in_maps, core_ids=list(range(N_CORES)), trace=TRACE
    )
    _CACHED["last_results"] = res
    return np.concatenate(
        [res.results[c]["out"] for c in range(N_CORES)], axis=0
    )
